# revision 6
# baseline (speedup 1.0000x reference)
"""Trainium2 Bass kernel: complex nearest-neighbor 2x2 upsampling.

y[b, i, j, c] = complex(x_re, x_im)[b, i//2, j//2, c]
  inputs : x_re, x_im  f32 [16, 128, 128, 64]
  output : complex64   [16, 256, 256, 64]

Data-parallel over batch: 2 examples per core on 8 cores. Per core the kernel
is pure data movement (16 MiB in + 64 MiB out):
  - partition dim = h (128 rows)
  - load full-example re/im planes with 4 MiB DMAs (32 KiB/partition lines)
    on the ACT HWDGE ring
  - DVE (re) + ACT (im) copies build the complex-interleaved, w-duplicated
    rows in SBUF (broadcast APs do the duplication)
  - stores on the Sync HWDGE ring write fully contiguous 64 KiB/partition
    lines; row duplication (i = 2h, 2h+1) comes from storing each tile twice
    (or once with a 0-stride repeat AP)
"""
import numpy as np

import concourse.bass as bass
import concourse.tile as tile
from concourse import bacc, mybir
from concourse import bass_utils

# Full-problem constants (hardcoded per harness contract)
B, H, W, C = 16, 128, 128, 64
N_CORES = 8
B_SHARD = B // N_CORES  # 2 examples per core

_CACHE = {}

# default configuration (best measured: ~208us/core near-solo, ~245-250us when
# the HBM-stack-mate core fully overlaps; chip roofline for 8x(16MiB in +
# 64MiB out) at ~2.9TB/s is ~234us).
#
# Measured HW model (2026-08 session): ALL of one core's DMA traffic — SWDGE
# (gpsimd) and HWDGE (sync/scalar) queues alike — drains through ONE shared
# ~430-435 GB/s pipe (16 SDMA engines, packet-granular round-robin between
# queues; per-20us trace bins show loads+stores summing to a constant
# ~430-440 GB/s whatever the queue split). Exec time ~= 9.2us fixed preamble
# + 80MiB/430GB/s (~195us) + ~2-8us tail. Dead ends, all HW-measured:
#   - dtype-cast DMA (gpsimd bf16<->f32) is charged at the f32-side bytes
#     (cast store 8MiB-f32/4MiB-bf16 took exactly a plain 8MiB store's time),
#     so bf16-in-SBUF does not shrink the pipe bytes;
#   - chunked/paced loads + raw-bacc small-semaphore pipelines (v3/v4/v5
#     below) start the first store at 14-18us instead of 40us and cut the
#     Tile epilogue from ~8.6us to ~2.5us, but lose the same amount to less
#     efficient small-chunk load packets -> all land at 209-213us vs 207.9.
CFG = dict(wc=32, full_b_loads=True, load_engine="gpsimd", store_repeat=False,
           inp_bufs=2, out_bufs=2)


def build_nc(cfg=None):
    """Build and compile the per-core Bass module (B_SHARD examples)."""
    cfg = {**CFG, **(cfg or {})}
    wc = cfg["wc"]
    nc = bacc.Bacc("TRN2", debug=False, num_devices=N_CORES)
    x_re = nc.dram_tensor(
        "x_re", [B_SHARD, H, W, C], mybir.dt.float32, kind="ExternalInput"
    ).ap()
    x_im = nc.dram_tensor(
        "x_im", [B_SHARD, H, W, C], mybir.dt.float32, kind="ExternalInput"
    ).ap()
    # f32 view of the complex64 output: last dim is (c, comp) interleaved
    y = nc.dram_tensor(
        "y", [B_SHARD, 2 * H, 2 * W, 2 * C], mybir.dt.float32, kind="ExternalOutput"
    ).ap()

    load = getattr(nc, cfg["load_engine"]).dma_start

    with tile.TileContext(nc) as tc:
        with (
            tc.tile_pool(name="inp", bufs=cfg["inp_bufs"]) as inp,
            tc.tile_pool(name="outp", bufs=cfg["out_bufs"]) as outp,
        ):
            for b in range(B_SHARD):
                if cfg["full_b_loads"]:
                    re_t = inp.tile([H, W * C], mybir.dt.float32, tag="re")
                    load(re_t[:], x_re[b].rearrange("h w c -> h (w c)"))
                    im_t = inp.tile([H, W * C], mybir.dt.float32, tag="im")
                    load(im_t[:], x_im[b].rearrange("h w c -> h (w c)"))
                for wi in range(W // wc):
                    if not cfg["full_b_loads"]:
                        re_t = inp.tile([H, wc * C], mybir.dt.float32, tag="re")
                        load(re_t[:], x_re[b, :, wi * wc:(wi + 1) * wc, :]
                             .rearrange("h w c -> h (w c)"))
                        im_t = inp.tile([H, wc * C], mybir.dt.float32, tag="im")
                        load(im_t[:], x_im[b, :, wi * wc:(wi + 1) * wc, :]
                             .rearrange("h w c -> h (w c)"))
                        sl = slice(0, wc * C)
                    else:
                        sl = slice(wi * wc * C, (wi + 1) * wc * C)
                    cplx = outp.tile([H, wc * 2 * C * 2], mybir.dt.float32, tag="cplx")
                    dst5 = cplx[:].rearrange(
                        "p (w dup c comp) -> p w dup c comp", w=wc, dup=2, c=C, comp=2
                    )
                    src_re = (re_t[:, sl].rearrange("p (w c) -> p w c", w=wc)
                              .unsqueeze(2).broadcast_to([H, wc, 2, C]))
                    src_im = (im_t[:, sl].rearrange("p (w c) -> p w c", w=wc)
                              .unsqueeze(2).broadcast_to([H, wc, 2, C]))
                    nc.vector.tensor_copy(dst5[:, :, :, :, 0], src_re)
                    nc.scalar.copy(dst5[:, :, :, :, 1], src_im)
                    if cfg["store_repeat"]:
                        dst = y[b, :, 2 * wi * wc:2 * (wi + 1) * wc, :].rearrange(
                            "(h r) j cc -> h r (j cc)", r=2
                        )
                        src = cplx[:].unsqueeze(1).broadcast_to(
                            [H, 2, wc * 2 * C * 2]
                        )
                        nc.sync.dma_start(dst, src)
                    else:
                        for r in range(2):
                            nc.sync.dma_start(
                                y[b, r::2, 2 * wi * wc:2 * (wi + 1) * wc, :]
                                .rearrange("i j cc -> i (j cc)"),
                                cplx[:],
                            )
    nc.compile()
    return nc


def build_nc_raw(wc=32):
    """Raw-bacc variant: manual semaphores, no TileContext pre/post barriers.

    Same dataflow as the Tile version (gpsimd loads, DVE/ACT interleave,
    sync stores, cplx double buffer). Every semaphore has at most one
    outstanding DMA, so all wait values are unambiguous under any
    completion interleaving (validated by CoreSim's race detector).
    """
    from contextlib import ExitStack

    nc = bacc.Bacc("TRN2", debug=False, num_devices=N_CORES)
    x_re = nc.dram_tensor(
        "x_re", [B_SHARD, H, W, C], mybir.dt.float32, kind="ExternalInput"
    ).ap()
    x_im = nc.dram_tensor(
        "x_im", [B_SHARD, H, W, C], mybir.dt.float32, kind="ExternalInput"
    ).ap()
    y = nc.dram_tensor(
        "y", [B_SHARD, 2 * H, 2 * W, 2 * C], mybir.dt.float32, kind="ExternalOutput"
    ).ap()
    NCH = W // wc
    NK = B_SHARD * NCH
    with ExitStack() as ctx:
        block = ctx.enter_context(nc.Block())
        re_sem = [ctx.enter_context(nc.semaphore(f"re_sem{b}")) for b in range(B_SHARD)]
        im_sem = [ctx.enter_context(nc.semaphore(f"im_sem{b}")) for b in range(B_SHARD)]
        st_sem = [[ctx.enter_context(nc.semaphore(f"st_sem{p}{r}")) for r in range(2)]
                  for p in range(2)]
        vsem = ctx.enter_context(nc.semaphore("vsem"))
        ssem = ctx.enter_context(nc.semaphore("ssem"))
        re = [ctx.enter_context(nc.sbuf_tensor(f"re{b}", [H, W * C], mybir.dt.float32))
              for b in range(B_SHARD)]
        im = [ctx.enter_context(nc.sbuf_tensor(f"im{b}", [H, W * C], mybir.dt.float32))
              for b in range(B_SHARD)]
        cplx = [ctx.enter_context(
            nc.sbuf_tensor(f"cplx{p}", [H, wc * 2 * C * 2], mybir.dt.float32))
            for p in range(2)]

        @block.gpsimd
        def _(g):
            for b in range(B_SHARD):
                g.dma_start(re[b][:], x_re[b].rearrange("h w c -> h (w c)")
                            ).then_inc(re_sem[b], 16)
                g.dma_start(im[b][:], x_im[b].rearrange("h w c -> h (w c)")
                            ).then_inc(im_sem[b], 16)

        def dst5(k):
            return cplx[k % 2][:].rearrange(
                "p (w dup c comp) -> p w dup c comp", w=wc, dup=2, c=C, comp=2)

        def srcv(t, b, wi):
            sl = slice(wi * wc * C, (wi + 1) * wc * C)
            return (t[b][:, sl].rearrange("p (w c) -> p w c", w=wc)
                    .unsqueeze(2).broadcast_to([H, wc, 2, C]))

        @block.vector
        def _(v):
            for k in range(NK):
                b, wi = divmod(k, NCH)
                v.wait_ge(re_sem[b], 16)
                if k >= 2:
                    v.wait_ge(st_sem[k % 2][0], 16 * (k // 2))
                    v.wait_ge(st_sem[k % 2][1], 16 * (k // 2))
                nc.vector.tensor_copy(dst5(k)[:, :, :, :, 0], srcv(re, b, wi)
                                      ).then_inc(vsem, 1)

        @block.scalar
        def _(s):
            for k in range(NK):
                b, wi = divmod(k, NCH)
                s.wait_ge(im_sem[b], 16)
                if k >= 2:
                    s.wait_ge(st_sem[k % 2][0], 16 * (k // 2))
                    s.wait_ge(st_sem[k % 2][1], 16 * (k // 2))
                nc.scalar.copy(dst5(k)[:, :, :, :, 1], srcv(im, b, wi)
                               ).then_inc(ssem, 1)

        @block.sync
        def _(sy):
            for k in range(NK):
                b, wi = divmod(k, NCH)
                sy.wait_ge(vsem, k + 1)
                sy.wait_ge(ssem, k + 1)
                for r in range(2):
                    sy.dma_start(
                        y[b, r::2, 2 * wi * wc:2 * (wi + 1) * wc, :]
                        .rearrange("i j cc -> i (j cc)"),
                        cplx[k % 2][:],
                    ).then_inc(st_sem[k % 2][r], 16)
            for p in range(2):
                for r in range(2):
                    sy.wait_ge(st_sem[p][r], 16 * (NK // 2))
    nc.compile()
    return nc


def build_nc_v3(wcs=(8, 24, 32, 32, 32), ib=3, cb=3):
    """Raw-bacc pipelined variant: chunked loads, minimal semaphores.

    Per example the W dim is processed in chunks of wcs (first chunk small so
    the first store launches ~5us after the preamble). Loads on gpsimd
    (SWDGE, its own SDMA engine pool), re-interleave on DVE, im on ACT,
    stores on sync (HWDGE). Slot-granular semaphores: every sem has at most
    one outstanding DMA, so wait values are unambiguous.
    """
    from contextlib import ExitStack

    assert sum(wcs) == W
    wmax = max(wcs)
    chunks = [(b, int(w0), int(wc))
              for b in range(B_SHARD)
              for w0, wc in zip(np.cumsum((0,) + tuple(wcs[:-1])), wcs)]
    K = len(chunks)

    nc = bacc.Bacc("TRN2", debug=False, num_devices=N_CORES)
    x_re = nc.dram_tensor(
        "x_re", [B_SHARD, H, W, C], mybir.dt.float32, kind="ExternalInput"
    ).ap()
    x_im = nc.dram_tensor(
        "x_im", [B_SHARD, H, W, C], mybir.dt.float32, kind="ExternalInput"
    ).ap()
    y = nc.dram_tensor(
        "y", [B_SHARD, 2 * H, 2 * W, 2 * C], mybir.dt.float32, kind="ExternalOutput"
    ).ap()

    with ExitStack() as ctx:
        block = ctx.enter_context(nc.Block())
        re_s = [ctx.enter_context(nc.semaphore(f"re_s{s}")) for s in range(ib)]
        im_s = [ctx.enter_context(nc.semaphore(f"im_s{s}")) for s in range(ib)]
        st_s = [[ctx.enter_context(nc.semaphore(f"st_s{c}_{r}")) for r in range(2)]
                for c in range(cb)]
        vsem = ctx.enter_context(nc.semaphore("vsem"))
        ssem = ctx.enter_context(nc.semaphore("ssem"))
        re_t = [ctx.enter_context(
            nc.sbuf_tensor(f"re{s}", [H, wmax * C], mybir.dt.float32))
            for s in range(ib)]
        im_t = [ctx.enter_context(
            nc.sbuf_tensor(f"im{s}", [H, wmax * C], mybir.dt.float32))
            for s in range(ib)]
        cplx = [ctx.enter_context(
            nc.sbuf_tensor(f"cplx{c}", [H, wmax * 2 * C * 2], mybir.dt.float32))
            for c in range(cb)]

        @block.gpsimd
        def _(g):
            for k, (b, w0, wc) in enumerate(chunks):
                s = k % ib
                if k >= ib:
                    g.wait_ge(vsem, k - ib + 1)
                    g.wait_ge(ssem, k - ib + 1)
                g.dma_start(
                    re_t[s][:, :wc * C],
                    x_re[b, :, w0:w0 + wc, :].rearrange("h w c -> h (w c)"),
                ).then_inc(re_s[s], 16)
                g.dma_start(
                    im_t[s][:, :wc * C],
                    x_im[b, :, w0:w0 + wc, :].rearrange("h w c -> h (w c)"),
                ).then_inc(im_s[s], 16)

        def dst5(k, wc):
            return cplx[k % cb][:, :wc * 2 * C * 2].rearrange(
                "p (w dup c comp) -> p w dup c comp", w=wc, dup=2, c=C, comp=2)

        def srcv(t, k, wc):
            return (t[k % ib][:, :wc * C].rearrange("p (w c) -> p w c", w=wc)
                    .unsqueeze(2).broadcast_to([H, wc, 2, C]))

        @block.vector
        def _(v):
            for k, (b, w0, wc) in enumerate(chunks):
                v.wait_ge(re_s[k % ib], 16 * (k // ib + 1))
                if k >= cb:
                    for r in range(2):
                        v.wait_ge(st_s[k % cb][r], 16 * (k // cb))
                nc.vector.tensor_copy(
                    dst5(k, wc)[:, :, :, :, 0], srcv(re_t, k, wc)
                ).then_inc(vsem, 1)

        @block.scalar
        def _(s_):
            for k, (b, w0, wc) in enumerate(chunks):
                s_.wait_ge(im_s[k % ib], 16 * (k // ib + 1))
                if k >= cb:
                    for r in range(2):
                        s_.wait_ge(st_s[k % cb][r], 16 * (k // cb))
                nc.scalar.copy(
                    dst5(k, wc)[:, :, :, :, 1], srcv(im_t, k, wc)
                ).then_inc(ssem, 1)

        @block.sync
        def _(sy):
            for k, (b, w0, wc) in enumerate(chunks):
                sy.wait_ge(vsem, k + 1)
                sy.wait_ge(ssem, k + 1)
                for r in range(2):
                    sy.dma_start(
                        y[b, r::2, 2 * w0:2 * (w0 + wc), :]
                        .rearrange("i j cc -> i (j cc)"),
                        cplx[k % cb][:, :wc * 2 * C * 2],
                    ).then_inc(st_s[k % cb][r], 16)
            for c in range(cb):
                uses = len([k for k in range(K) if k % cb == c])
                for r in range(2):
                    sy.wait_ge(st_s[c][r], 16 * uses)
    nc.compile()
    return nc


def build_nc_v4(wcs=(8, 24, 32, 32, 32), cb=2):
    """Raw pipelined variant, front-loaded loads.

    All loads dispatch immediately into dedicated per-chunk buffers (16 MiB
    SBUF = 128 KiB/partition) so the HBM read burst completes early and the
    long store stream then owns the stack. First chunk is small so the first
    store launches right after the preamble. Loads on gpsimd (SWDGE pool),
    re-copy DVE, im-copy ACT, stores on sync (HWDGE pool).
    """
    from contextlib import ExitStack

    assert sum(wcs) == W
    chunks = [(b, int(w0), int(wc))
              for b in range(B_SHARD)
              for w0, wc in zip(np.cumsum((0,) + tuple(wcs[:-1])), wcs)]
    K = len(chunks)

    nc = bacc.Bacc("TRN2", debug=False, num_devices=N_CORES)
    x_re = nc.dram_tensor(
        "x_re", [B_SHARD, H, W, C], mybir.dt.float32, kind="ExternalInput"
    ).ap()
    x_im = nc.dram_tensor(
        "x_im", [B_SHARD, H, W, C], mybir.dt.float32, kind="ExternalInput"
    ).ap()
    y = nc.dram_tensor(
        "y", [B_SHARD, 2 * H, 2 * W, 2 * C], mybir.dt.float32, kind="ExternalOutput"
    ).ap()

    with ExitStack() as ctx:
        block = ctx.enter_context(nc.Block())
        re_s = [ctx.enter_context(nc.semaphore(f"re_s{k}")) for k in range(K)]
        im_s = [ctx.enter_context(nc.semaphore(f"im_s{k}")) for k in range(K)]
        st_s = [[ctx.enter_context(nc.semaphore(f"st_s{c}_{r}")) for r in range(2)]
                for c in range(cb)]
        vsem = ctx.enter_context(nc.semaphore("vsem"))
        ssem = ctx.enter_context(nc.semaphore("ssem"))
        re_t = [ctx.enter_context(
            nc.sbuf_tensor(f"re{k}", [H, wc * C], mybir.dt.float32))
            for k, (b, w0, wc) in enumerate(chunks)]
        im_t = [ctx.enter_context(
            nc.sbuf_tensor(f"im{k}", [H, wc * C], mybir.dt.float32))
            for k, (b, w0, wc) in enumerate(chunks)]
        wmax = max(wcs)
        cplx = [ctx.enter_context(
            nc.sbuf_tensor(f"cplx{c}", [H, wmax * 2 * C * 2], mybir.dt.float32))
            for c in range(cb)]

        @block.gpsimd
        def _(g):
            for k, (b, w0, wc) in enumerate(chunks):
                g.dma_start(
                    re_t[k][:],
                    x_re[b, :, w0:w0 + wc, :].rearrange("h w c -> h (w c)"),
                ).then_inc(re_s[k], 16)
                g.dma_start(
                    im_t[k][:],
                    x_im[b, :, w0:w0 + wc, :].rearrange("h w c -> h (w c)"),
                ).then_inc(im_s[k], 16)

        def dst5(k, wc):
            return cplx[k % cb][:, :wc * 2 * C * 2].rearrange(
                "p (w dup c comp) -> p w dup c comp", w=wc, dup=2, c=C, comp=2)

        def srcv(t, k, wc):
            return (t[k][:].rearrange("p (w c) -> p w c", w=wc)
                    .unsqueeze(2).broadcast_to([H, wc, 2, C]))

        @block.vector
        def _(v):
            for k, (b, w0, wc) in enumerate(chunks):
                v.wait_ge(re_s[k], 16)
                if k >= cb:
                    for r in range(2):
                        v.wait_ge(st_s[k % cb][r], 16 * (k // cb))
                nc.vector.tensor_copy(
                    dst5(k, wc)[:, :, :, :, 0], srcv(re_t, k, wc)
                ).then_inc(vsem, 1)

        @block.scalar
        def _(s_):
            for k, (b, w0, wc) in enumerate(chunks):
                s_.wait_ge(im_s[k], 16)
                if k >= cb:
                    for r in range(2):
                        s_.wait_ge(st_s[k % cb][r], 16 * (k // cb))
                nc.scalar.copy(
                    dst5(k, wc)[:, :, :, :, 1], srcv(im_t, k, wc)
                ).then_inc(ssem, 1)

        @block.sync
        def _(sy):
            for k, (b, w0, wc) in enumerate(chunks):
                sy.wait_ge(vsem, k + 1)
                sy.wait_ge(ssem, k + 1)
                for r in range(2):
                    sy.dma_start(
                        y[b, r::2, 2 * w0:2 * (w0 + wc), :]
                        .rearrange("i j cc -> i (j cc)"),
                        cplx[k % cb][:, :wc * 2 * C * 2],
                    ).then_inc(st_s[k % cb][r], 16)
            for c in range(cb):
                uses = len([k for k in range(K) if k % cb == c])
                for r in range(2):
                    sy.wait_ge(st_s[c][r], 16 * uses)
    nc.compile()
    return nc


def build_nc_v5(cb=2):
    """Raw pipelined variant: small first chunk + big bulk loads.

    Load plan (all dispatched up-front on gpsimd/SWDGE, dedicated buffers):
      re/im[b0, :, 0:8]    256 KiB each  -> first store launches ~14us
      re/im[b0, :, 8:128]  3.75 MiB each (30 KiB partition lines, line-rate)
      re/im[b1]            4 MiB each    (32 KiB partition lines)
    Interleave chunks read slices of those tiles; stores on sync (HWDGE).
    The single ~435 GB/s DMA pipe stays packed from ~9us to the last store.
    """
    from contextlib import ExitStack

    W0 = 8  # first-chunk width
    # (b, w0, wc, tile, off) — tile 0: b0 small, 1: b0 rest, 2: b1 full
    chunks = [(0, 0, 8, 0, 0), (0, 8, 32, 1, 0), (0, 40, 32, 1, 32),
              (0, 72, 32, 1, 64), (0, 104, 24, 1, 96),
              (1, 0, 32, 2, 0), (1, 32, 32, 2, 32), (1, 64, 32, 2, 64),
              (1, 96, 32, 2, 96)]
    K = len(chunks)
    tile_w = [W0, W - W0, W]
    loads = [  # (tile, b, wlo, whi)
        (0, 0, 0, W0), (1, 0, W0, W), (2, 1, 0, W)]

    nc = bacc.Bacc("TRN2", debug=False, num_devices=N_CORES)
    x_re = nc.dram_tensor(
        "x_re", [B_SHARD, H, W, C], mybir.dt.float32, kind="ExternalInput"
    ).ap()
    x_im = nc.dram_tensor(
        "x_im", [B_SHARD, H, W, C], mybir.dt.float32, kind="ExternalInput"
    ).ap()
    y = nc.dram_tensor(
        "y", [B_SHARD, 2 * H, 2 * W, 2 * C], mybir.dt.float32, kind="ExternalOutput"
    ).ap()

    with ExitStack() as ctx:
        block = ctx.enter_context(nc.Block())
        re_s = [ctx.enter_context(nc.semaphore(f"re_s{t}")) for t in range(3)]
        im_s = [ctx.enter_context(nc.semaphore(f"im_s{t}")) for t in range(3)]
        st_s = [[ctx.enter_context(nc.semaphore(f"st_s{c}_{r}")) for r in range(2)]
                for c in range(cb)]
        vsem = ctx.enter_context(nc.semaphore("vsem"))
        ssem = ctx.enter_context(nc.semaphore("ssem"))
        re_t = [ctx.enter_context(
            nc.sbuf_tensor(f"re{t}", [H, tw * C], mybir.dt.float32))
            for t, tw in enumerate(tile_w)]
        im_t = [ctx.enter_context(
            nc.sbuf_tensor(f"im{t}", [H, tw * C], mybir.dt.float32))
            for t, tw in enumerate(tile_w)]
        cplx = [ctx.enter_context(
            nc.sbuf_tensor(f"cplx{c}", [H, 32 * 2 * C * 2], mybir.dt.float32))
            for c in range(cb)]

        @block.gpsimd
        def _(g):
            for t, b, wlo, whi in loads:
                g.dma_start(
                    re_t[t][:],
                    x_re[b, :, wlo:whi, :].rearrange("h w c -> h (w c)"),
                ).then_inc(re_s[t], 16)
                g.dma_start(
                    im_t[t][:],
                    x_im[b, :, wlo:whi, :].rearrange("h w c -> h (w c)"),
                ).then_inc(im_s[t], 16)

        def dst5(k, wc):
            return cplx[k % cb][:, :wc * 2 * C * 2].rearrange(
                "p (w dup c comp) -> p w dup c comp", w=wc, dup=2, c=C, comp=2)

        def srcv(tiles, t, off, wc):
            return (tiles[t][:, off * C:(off + wc) * C]
                    .rearrange("p (w c) -> p w c", w=wc)
                    .unsqueeze(2).broadcast_to([H, wc, 2, C]))

        @block.vector
        def _(v):
            for k, (b, w0, wc, t, off) in enumerate(chunks):
                v.wait_ge(re_s[t], 16)
                if k >= cb:
                    for r in range(2):
                        v.wait_ge(st_s[k % cb][r], 16 * (k // cb))
                nc.vector.tensor_copy(
                    dst5(k, wc)[:, :, :, :, 0], srcv(re_t, t, off, wc)
                ).then_inc(vsem, 1)

        @block.scalar
        def _(s_):
            for k, (b, w0, wc, t, off) in enumerate(chunks):
                s_.wait_ge(im_s[t], 16)
                if k >= cb:
                    for r in range(2):
                        s_.wait_ge(st_s[k % cb][r], 16 * (k // cb))
                nc.scalar.copy(
                    dst5(k, wc)[:, :, :, :, 1], srcv(im_t, t, off, wc)
                ).then_inc(ssem, 1)

        @block.sync
        def _(sy):
            for k, (b, w0, wc, t, off) in enumerate(chunks):
                sy.wait_ge(vsem, k + 1)
                sy.wait_ge(ssem, k + 1)
                for r in range(2):
                    sy.dma_start(
                        y[b, r::2, 2 * w0:2 * (w0 + wc), :]
                        .rearrange("i j cc -> i (j cc)"),
                        cplx[k % cb][:, :wc * 2 * C * 2],
                    ).then_inc(st_s[k % cb][r], 16)
            for c in range(cb):
                uses = len([k for k in range(K) if k % cb == c])
                for r in range(2):
                    sy.wait_ge(st_s[c][r], 16 * uses)
    nc.compile()
    return nc


def _get_nc(cfg=None):
    merged = {**CFG, **(cfg or {})}
    key = str(sorted(merged.items()))
    if key not in _CACHE:
        if merged.get("v5"):
            _CACHE[key] = build_nc_v5(cb=merged.get("out_bufs", 2))
        elif merged.get("v4"):
            _CACHE[key] = build_nc_v4(
                wcs=merged.get("wcs", (8, 24, 32, 32, 32)),
                cb=merged.get("out_bufs", 2))
        elif merged.get("v3"):
            _CACHE[key] = build_nc_v3(
                wcs=merged.get("wcs", (8, 24, 32, 32, 32)),
                ib=merged.get("inp_bufs", 3), cb=merged.get("out_bufs", 3))
        elif merged.get("raw"):
            _CACHE[key] = build_nc_raw(wc=merged["wc"])
        else:
            _CACHE[key] = build_nc(cfg)
    return _CACHE[key]


def run_sharded(x_re, x_im, trace=False, cfg=None):
    """Run the SPMD kernel; returns (full complex64 output, BassKernelResults)."""
    nc = _get_nc(cfg)
    in_maps = [
        {
            "x_re": np.ascontiguousarray(x_re[m * B_SHARD:(m + 1) * B_SHARD]),
            "x_im": np.ascontiguousarray(x_im[m * B_SHARD:(m + 1) * B_SHARD]),
        }
        for m in range(N_CORES)
    ]
    res = bass_utils.run_bass_kernel_spmd(
        nc, in_maps, core_ids=list(range(N_CORES)), trace=trace
    )
    parts = [res.results[m]["y"] for m in range(N_CORES)]
    out_f32 = np.concatenate(parts, axis=0)  # [16, 256, 256, 128] f32
    out = out_f32.view(np.complex64)  # [16, 256, 256, 64] c64
    return out, res


def kernel(x_re, x_im):
    x_re = np.asarray(x_re, dtype=np.float32)
    x_im = np.asarray(x_im, dtype=np.float32)
    out, _ = run_sharded(x_re, x_im, trace=False)
    return out

